# revision 14
# baseline (speedup 1.0000x reference)
"""CausalGraphVAE on 8 Trainium2 NeuronCores (Bass/Tile, SPMD).

Distribution (1D row-parallel, per the classic SpMM layout):
  Core c owns nodes J = [512c, 512c+512).
  - It loads edge_score[:, J] (adjacency COLUMN shard = anorm_t ROW shard),
    computes adj[:, J] = sigmoid(...) -> output shard, and the column sums
    deg[J] (fully local!), dinv[J] = 1/sqrt(deg[J]).
  - GCN aggregation out[J,:] = anorm_t[J,:] @ U is computed as
    dinv[J] (*) ( A_blk.T @ (dinv (*) U) ) where A_blk = adj[:, J]:
    every core scales ITS rows of U by its local dinv before the
    AllGather, so no dinv exchange is ever needed.
  - Dense GEMMs + gates run on the local 512-node shard in a
    feature-major layout ([F, nodes] on SBUF) so per-feature biases are
    per-partition and no transposes are needed until the final outputs.
  - 3 AllGathers per forward (one per TGCN block) move the [4096, 512]
    pre-aggregation features (bf16).

dtypes: big SpMM in bf16 (A block + gathered U), dense GEMMs in fp32r
(full-rate PE, ~1e-4 matmul error), everything else fp32.
"""

import os

import numpy as np
import jax
import jax.numpy as jnp

import concourse.mybir as mybir
import concourse.tile as tile
from concourse import bacc, bass2jax
from concourse.masks import make_identity

NCORES = 8
N = 4096
S = N // NCORES          # 512 nodes per core
P = 128
KT = N // P              # 32 node k-tiles
ST = S // P              # 4 shard m-tiles
DIN = 256
HID = 256
LAT = 64

F32 = mybir.dt.float32
F32R = mybir.dt.float32r
BF16 = mybir.dt.bfloat16
SPMM_DT = BF16           # dtype of A block + gathered U for the big SpMM
AF = mybir.ActivationFunctionType
ALU = mybir.AluOpType

# bias column map in the packed [128, 19] bias tensor
BE, BT, NZ1, GH1, NZ2, GH2, NZ3, GH3, BMULV, BD = 0, 2, 4, 6, 8, 10, 12, 14, 16, 17
NBIA = 19


def _build():
    nc = bacc.Bacc("TRN2", num_devices=NCORES)

    # ---------------- I/O ----------------
    ecol = nc.dram_tensor("ecol", [N, S], F32, kind="ExternalInput")
    xT = nc.dram_tensor("xT", [DIN, S], F32, kind="ExternalInput")
    eeT = nc.dram_tensor("eeT", [DIN, S], F32, kind="ExternalInput")
    ttT = nc.dram_tensor("ttT", [DIN, S], F32, kind="ExternalInput")
    we_i = nc.dram_tensor("we", [DIN, HID], F32, kind="ExternalInput")
    wt_i = nc.dram_tensor("wt", [DIN, HID], F32, kind="ExternalInput")
    w1_i = nc.dram_tensor("w1", [DIN + 2 * HID, 2 * HID], F32, kind="ExternalInput")
    lw1z_i = nc.dram_tensor("lw1z", [HID, HID], F32, kind="ExternalInput")
    lw1h_i = nc.dram_tensor("lw1h", [HID, HID], F32, kind="ExternalInput")
    w2_i = nc.dram_tensor("w2", [HID, 2 * HID], F32, kind="ExternalInput")
    lw2z_i = nc.dram_tensor("lw2z", [HID, HID], F32, kind="ExternalInput")
    lw2h_i = nc.dram_tensor("lw2h", [HID, HID], F32, kind="ExternalInput")
    wmulv_i = nc.dram_tensor("wmulv", [HID, 2 * LAT], F32, kind="ExternalInput")
    wd_i = nc.dram_tensor("wd", [P, HID], F32, kind="ExternalInput")  # zero padded
    w3_i = nc.dram_tensor("w3", [HID, 2 * DIN], F32, kind="ExternalInput")
    lw3z_i = nc.dram_tensor("lw3z", [DIN, DIN], F32, kind="ExternalInput")
    lw3h_i = nc.dram_tensor("lw3h", [DIN, DIN], F32, kind="ExternalInput")
    bia_i = nc.dram_tensor("bia", [P, NBIA], F32, kind="ExternalInput")
    epsT_i = nc.dram_tensor("epsT", [LAT, S], F32, kind="ExternalInput")

    adj_o = nc.dram_tensor("adj_o", [N, S], F32, kind="ExternalOutput")
    recon_o = nc.dram_tensor("recon_o", [S, DIN], F32, kind="ExternalOutput")
    mu_o = nc.dram_tensor("mu_o", [S, LAT], F32, kind="ExternalOutput")
    lv_o = nc.dram_tensor("lv_o", [S, LAT], F32, kind="ExternalOutput")
    dbg = {
        name: nc.dram_tensor(f"dbg_{name}", [P, S], F32, kind="ExternalOutput")
        for name in ("dinv", "u1", "cz1", "h1", "h2", "z", "xd", "cz3", "zc3",
                     "ht3", "rec", "deg", "rcp")
    }

    def dump(name, ap):
        eng = nc.sync if ap.dtype == F32 else nc.gpsimd
        eng.dma_start(dbg[name][: ap.shape[0], : ap.shape[-1]], ap)

    with tile.TileContext(nc) as tc:
        with (
            tc.tile_pool(name="pres", bufs=1) as pres,      # resident tiles
            tc.tile_pool(name="stg", bufs=4) as stg,        # sigmoid staging
            tc.tile_pool(name="wstg", bufs=2) as wstg,      # weight fp32 staging
            tc.tile_pool(name="upool", bufs=6) as upool,    # streamed U k-tiles
            tc.tile_pool(name="uev", bufs=3) as uev,        # U eviction tiles
            tc.tile_pool(name="ostg", bufs=3) as ostg,      # output transpose staging
            tc.tile_pool(name="psA", bufs=4, space="PSUM") as psA,
            tc.tile_pool(name="psD", bufs=1, space="PSUM") as psD,
            tc.tile_pool(name="psT", bufs=2, space="PSUM") as psT,
            tc.tile_pool(name="dram", bufs=1, space="DRAM") as dram,
        ):
            # ---------------- small constants ----------------
            bia_sb = pres.tile([P, NBIA], F32)
            nc.sync.dma_start(bia_sb[:], bia_i[:])
            epsT_sb = pres.tile([P, S], F32)
            nc.sync.dma_start(epsT_sb[:LAT], epsT_i[:])
            ident = pres.tile([P, P], F32)
            make_identity(nc, ident[:])
            ones_col = pres.tile([P, 1], F32)
            nc.vector.memset(ones_col[:], 1.0)
            ones_row = pres.tile([1, P], F32)
            nc.vector.memset(ones_row[:], 1.0)

            # ---------------- weights: DMA fp32, convert to fp32r ----------------
            def ldw(ap, kt, width, name):
                wr = pres.tile([P, kt, width], F32R, name=name)
                v = ap.rearrange("(t p) w -> t p w", p=P)
                for t in range(kt):
                    s = wstg.tile([P, width], F32, name="wstg", tag="wstg")
                    nc.sync.dma_start(s[:], v[t])
                    nc.vector.tensor_copy(wr[:, t], s[:])
                return wr

            we_r = ldw(we_i, 2, HID, "we_r")
            wt_r = ldw(wt_i, 2, HID, "wt_r")
            w1_r = ldw(w1_i, 6, 2 * HID, "w1_r")
            lw1z_r = ldw(lw1z_i, 2, HID, "lw1z_r")
            lw1h_r = ldw(lw1h_i, 2, HID, "lw1h_r")
            w2_r = ldw(w2_i, 2, 2 * HID, "w2_r")
            lw2z_r = ldw(lw2z_i, 2, HID, "lw2z_r")
            lw2h_r = ldw(lw2h_i, 2, HID, "lw2h_r")
            wmulv_r = ldw(wmulv_i, 2, 2 * LAT, "wmulv_r")
            wd_r = ldw(wd_i, 1, HID, "wd_r")
            w3_r = ldw(w3_i, 2, 2 * DIN, "w3_r")
            lw3z_r = ldw(lw3z_i, 2, DIN, "lw3z_r")
            lw3h_r = ldw(lw3h_i, 2, DIN, "lw3h_r")

            # embeddings (f-major shards) -> fp32r
            def ldx(ap, name):
                xr = pres.tile([P, 2, S], F32R, name=name)
                v = ap.rearrange("(t p) w -> t p w", p=P)
                for t in range(2):
                    s = wstg.tile([P, S], F32, name="xstg", tag="xstg")
                    nc.sync.dma_start(s[:], v[t])
                    nc.vector.tensor_copy(xr[:, t], s[:])
                return xr

            eeT_r = ldx(eeT, "eeT_r")
            ttT_r = ldx(ttT, "ttT_r")

            # ---------------- adjacency: sigmoid + adj out + deg + bf16 A ----------------
            A_r = pres.tile([P, KT, S], SPMM_DT)
            deg_ps = psD.tile([P, ST], F32)
            ec_t = ecol.rearrange("(t p) n -> t p n", p=P)
            ao_t = adj_o.rearrange("(t p) n -> t p n", p=P)
            for t in range(KT):
                s = stg.tile([P, S], F32, name="sg", tag="sg")
                nc.sync.dma_start(s[:], ec_t[t])
                a = stg.tile([P, S], F32, name="ad", tag="ad")
                nc.scalar.activation(a[:], s[:], AF.Sigmoid)
                nc.sync.dma_start(ao_t[t], a[:])
                nc.vector.tensor_copy(A_r[:, t], a[:])
                for m in range(ST):
                    # start only on the very first matmul: start=True clears
                    # the whole PSUM bank, not just this column's elements
                    nc.tensor.matmul(
                        deg_ps[:, m : m + 1],
                        a[:, P * m : P * (m + 1)],
                        ones_col[:],
                        start=(t == 0 and m == 0),
                        stop=(t == KT - 1),
                    )

            # dinv = 1/sqrt(deg)  (exact deg; DVE reciprocal + ACT sqrt)
            deg_sb = pres.tile([P, ST], F32)
            nc.vector.tensor_copy(deg_sb[:], deg_ps[:])
            rec_sb = pres.tile([P, ST], F32)
            nc.vector.reciprocal(rec_sb[:], deg_sb[:])
            dump("deg", deg_sb[:])
            dump("rcp", rec_sb[:])
            dinv_sb = pres.tile([P, ST], F32)
            nc.scalar.activation(dinv_sb[:], rec_sb[:], AF.Sqrt)
            dump("dinv", dinv_sb[:])

            # dinv broadcast tile [128, S]: DRAM round trip to a row, then
            # ones[1,128] (x) row[1,S] K=1 matmul
            dd = dram.tile([S], F32)
            nc.sync.dma_start(dd[:].rearrange("(m p) -> p m", p=P), dinv_sb[:])
            dinv_row = pres.tile([1, S], F32)
            nc.sync.dma_start(dinv_row[:], dd[:].rearrange("(o s) -> o s", o=1))
            bc_ps = psA.tile([P, S], F32, name="acc", tag="acc")
            nc.tensor.matmul(bc_ps[:], ones_row[:], dinv_row[:], start=True, stop=True)
            dinv_bc = pres.tile([P, S], F32)
            nc.vector.tensor_copy(dinv_bc[:], bc_ps[:])

            # ---------------- h0 = [x, relu(ee@We+be), relu(tt@Wt+bt)] (f-major) ----------------
            h0_r = pres.tile([P, 6, S], F32R)
            xv = xT.rearrange("(t p) w -> t p w", p=P)
            for t in range(2):
                s = wstg.tile([P, S], F32, name="xstg", tag="xstg")
                nc.sync.dma_start(s[:], xv[t])
                nc.vector.tensor_copy(h0_r[:, t], s[:])
            for (emb_r, w_r, bcol, off) in ((eeT_r, we_r, BE, 2), (ttT_r, wt_r, BT, 4)):
                for m in range(2):
                    ps = psA.tile([P, S], F32, name="acc", tag="acc")
                    for t in range(2):
                        nc.tensor.matmul(
                            ps[:],
                            w_r[:, t, P * m : P * (m + 1)],
                            emb_r[:, t],
                            start=(t == 0),
                            stop=(t == 1),
                        )
                    nc.scalar.activation(
                        h0_r[:, off + m], ps[:], AF.Relu,
                        bias=bia_sb[:, bcol + m : bcol + m + 1],
                    )

            # ---------------- one TGCN block ----------------
            def gcn_block(idx, lhs_r, lhs_kt, w_r, lwz_r, lwh_r, nz_col, gh_col,
                          fout, out_name, out_dtype):
                """lhs_r: f-major fp32r input [P, lhs_kt, S]; returns f-major
                relu((1-Z)*Ht) tile [P, fout//P, S] of out_dtype."""
                fo2 = 2 * fout // P  # psum m-tiles of the pre-agg GEMM (z|h concat)

                # U = lhs @ [Wz|Wh], node-major out, scaled by local dinv -> bf16
                ub = dram.tile([S, 2 * fout], SPMM_DT, name=f"ub{idx}")
                for m in range(ST):
                    ps = psA.tile([P, 2 * fout], F32, name="acc", tag="acc")
                    for t in range(lhs_kt):
                        nc.tensor.matmul(
                            ps[:],
                            lhs_r[:, t, P * m : P * (m + 1)],
                            w_r[:, t],
                            start=(t == 0),
                            stop=(t == lhs_kt - 1),
                        )
                    ue = uev.tile([P, 2 * fout], SPMM_DT, name="ue", tag="ue")
                    nc.vector.tensor_scalar(
                        ue[:], ps[:], dinv_sb[:, m : m + 1], None, ALU.mult
                    )
                    nc.sync.dma_start(ub[P * m : P * (m + 1), :], ue[:])
                    if idx == 1 and m == 0:
                        dump("u1", ue[:])

                uag = dram.tile([N, 2 * fout], SPMM_DT, name=f"uag{idx}",
                                addr_space="Shared")
                nc.gpsimd.collective_compute(
                    "AllGather",
                    ALU.bypass,
                    replica_groups=[list(range(NCORES))],
                    ins=[ub[:].opt()],
                    outs=[uag[:].opt()],
                )

                # SpMM: cz|ch (f-major) = (U_s).T-tiles @ A_blk, scaled by dinv_bc
                uag_t = uag[:].rearrange("(t p) n -> t p n", p=P)
                sps = [psA.tile([P, S], F32, name="acc", tag="acc")
                       for _ in range(fo2)]
                for t in range(KT):
                    ut = upool.tile([P, 2 * fout], SPMM_DT, name="ut", tag="ut")
                    nc.sync.dma_start(ut[:], uag_t[t])
                    for m in range(fo2):
                        nc.tensor.matmul(
                            sps[m],
                            ut[:, P * m : P * (m + 1)],
                            A_r[:, t],
                            start=(t == 0),
                            stop=(t == KT - 1),
                        )
                cz_r = pres.tile([P, fo2, S], F32R, name="cz_r", tag="cz_r",
                                 padded_shape=[P, 4, S])
                for m in range(fo2):
                    nc.vector.tensor_tensor(
                        cz_r[:, m], sps[m][:], dinv_bc[:], ALU.mult
                    )
                if idx == 1:
                    dump("cz1", cz_r[:, 0])
                if idx == 3:
                    dump("cz3", cz_r[:, 0])

                # gates: Zc = sigmoid(-(cz@lWz + gbz)), Ht = tanh(ch@lWh + gbh)
                fg = fout // P
                zc = pres.tile([P, fg, S], F32, name="zc", tag="zc",
                               padded_shape=[P, 2, S])
                ht = pres.tile([P, fg, S], F32, name="ht", tag="ht",
                               padded_shape=[P, 2, S])
                for (lw, dst, col, func, scl) in (
                    (lwz_r, zc, nz_col, AF.Sigmoid, -1.0),
                    (lwh_r, ht, gh_col, AF.Tanh, 1.0),
                ):
                    src_off = 0 if dst is zc else fg
                    for m in range(fg):
                        ps = psA.tile([P, S], F32, name="acc", tag="acc")
                        for t in range(fg):
                            nc.tensor.matmul(
                                ps[:],
                                lw[:, t, P * m : P * (m + 1)],
                                cz_r[:, src_off + t],
                                start=(t == 0),
                                stop=(t == fg - 1),
                            )
                        nc.scalar.activation(
                            dst[:, m], ps[:], func,
                            bias=bia_sb[:, col + m : col + m + 1], scale=scl,
                        )

                # out = relu(Zc * Ht)
                out = pres.tile([P, fg, S], out_dtype, name=out_name)
                for m in range(fg):
                    tmp = ostg.tile([P, S], F32, name="gtmp", tag="gtmp")
                    nc.vector.tensor_tensor(tmp[:], zc[:, m], ht[:, m], ALU.mult)
                    nc.vector.tensor_scalar(out[:, m], tmp[:], 0.0, None, ALU.max)
                if idx == 3:
                    dump("zc3", zc[:, 0])
                    dump("ht3", ht[:, 0])
                    dump("rec", out[:, 0])
                return out

            h1_r = gcn_block(1, h0_r, 6, w1_r, lw1z_r, lw1h_r, NZ1, GH1,
                             HID, "h1_r", F32R)
            dump("h1", h1_r[:, 0])
            h2_r = gcn_block(2, h1_r, 2, w2_r, lw2z_r, lw2h_r, NZ2, GH2,
                             HID, "h2_r", F32R)
            dump("h2", h2_r[:, 0])

            # ---------------- VAE head ----------------
            # mu|lv (f-major [128, S])
            ps = psA.tile([P, S], F32, name="acc", tag="acc")
            for t in range(2):
                nc.tensor.matmul(ps[:], wmulv_r[:, t], h2_r[:, t],
                                 start=(t == 0), stop=(t == 1))
            mulv_f = pres.tile([P, S], F32)
            nc.vector.tensor_scalar(
                mulv_f[:], ps[:], bia_sb[:, BMULV : BMULV + 1], None, ALU.add
            )

            # mu/lv outputs (transpose to node-major)
            for t in range(ST):
                pst = psT.tile([P, P], F32, name="tr", tag="tr")
                nc.tensor.transpose(pst[:], mulv_f[:, P * t : P * (t + 1)], ident[:])
                o = ostg.tile([P, P], F32, name="otr", tag="otr")
                nc.vector.tensor_copy(o[:], pst[:])
                nc.sync.dma_start(mu_o[P * t : P * (t + 1), :], o[:, :LAT])
                nc.sync.dma_start(lv_o[P * t : P * (t + 1), :], o[:, LAT:])

            # z = mu + eps * exp(0.5*lv)   (f-major, partitions 0:64)
            e5 = pres.tile([P, S], F32)
            nc.scalar.activation(e5[:LAT], mulv_f[LAT : 2 * LAT], AF.Exp, scale=0.5)
            z_r = pres.tile([P, S], F32R)
            nc.vector.tensor_scalar(z_r[:], dinv_bc[:], 0.0, None, ALU.mult)
            t1 = pres.tile([P, S], F32)
            nc.vector.tensor_tensor(t1[:LAT], epsT_sb[:LAT], e5[:LAT], ALU.mult)
            nc.vector.tensor_tensor(z_r[:LAT], t1[:LAT], mulv_f[:LAT], ALU.add)
            dump("z", z_r[:])

            # xd = z @ Wd + bd (f-major [256, S])
            xd_r = pres.tile([P, 2, S], F32R)
            for m in range(2):
                ps = psA.tile([P, S], F32, name="acc", tag="acc")
                nc.tensor.matmul(ps[:], wd_r[:, 0, P * m : P * (m + 1)], z_r[:],
                                 start=True, stop=True)
                nc.vector.tensor_scalar(
                    xd_r[:, m], ps[:], bia_sb[:, BD + m : BD + m + 1], None, ALU.add
                )
                if m == 0:
                    dump("xd", xd_r[:, 0])

            # ---------------- decoder TGCN -> recon ----------------
            recon_f = gcn_block(3, xd_r, 2, w3_r, lw3z_r, lw3h_r, NZ3, GH3,
                                DIN, "recon_f", F32)

            # recon output (transpose to node-major)
            for pt in range(2):
                for nt in range(ST):
                    pst = psT.tile([P, P], F32, name="tr", tag="tr")
                    nc.tensor.transpose(
                        pst[:], recon_f[:, pt, P * nt : P * (nt + 1)], ident[:]
                    )
                    o = ostg.tile([P, P], F32, name="otr", tag="otr")
                    nc.vector.tensor_copy(o[:], pst[:])
                    nc.sync.dma_start(
                        recon_o[P * nt : P * (nt + 1), P * pt : P * (pt + 1)], o[:]
                    )

    nc.finalize()
    return nc


# ---------------------------------------------------------------------------
# host side
# ---------------------------------------------------------------------------

_CACHE = {}


def _make_runner(nc):
    """Cached jit runner (mirrors bass2jax.run_bass_via_pjrt multi-core path,
    minus output-buffer donation so repeat calls don't re-upload zeros)."""
    bass2jax.install_neuronx_cc_hook()
    from jax.experimental.shard_map import shard_map
    from jax.sharding import Mesh, PartitionSpec

    partition_name = (
        nc.partition_id_tensor.name if nc.partition_id_tensor else None
    )
    in_names, out_names, out_avals, zero_outs = [], [], [], []
    for alloc in nc.m.functions[0].allocations:
        if not isinstance(alloc, mybir.MemoryLocationSet):
            continue
        name = alloc.memorylocations[0].name
        if alloc.kind == "ExternalInput":
            if name != partition_name:
                in_names.append(name)
        elif alloc.kind == "ExternalOutput":
            out_names.append(name)
            shape = tuple(alloc.tensor_shape)
            dtype = mybir.dt.np(alloc.dtype)
            out_avals.append(jax.core.ShapedArray(shape, dtype))
            zero_outs.append(np.zeros(shape, dtype))
    n_params = len(in_names)
    all_in_names = list(in_names) + out_names
    if partition_name is not None:
        all_in_names.append(partition_name)

    def _body(*args):
        operands = list(args)
        if partition_name is not None:
            operands.append(bass2jax.partition_id_tensor())
        outs = bass2jax._bass_exec_p.bind(
            *operands,
            out_avals=tuple(out_avals),
            in_names=tuple(all_in_names),
            out_names=tuple(out_names),
            lowering_input_output_aliases=(),
            sim_require_finite=True,
            sim_require_nnan=True,
            nc=nc,
        )
        return tuple(outs)

    from jax.sharding import NamedSharding

    devices = jax.devices()[:NCORES]
    mesh = Mesh(np.asarray(devices), ("core",))
    shd = NamedSharding(mesh, PartitionSpec("core"))
    nin = n_params + len(zero_outs)
    sharded = jax.jit(
        shard_map(
            _body,
            mesh=mesh,
            in_specs=(PartitionSpec("core"),) * nin,
            out_specs=(PartitionSpec("core"),) * len(out_names),
            check_rep=False,
        ),
        keep_unused=True,
    )
    zeros_cat = [
        jax.device_put(
            np.zeros((NCORES * z.shape[0], *z.shape[1:]), z.dtype), shd
        )
        for z in zero_outs
    ]

    class Runner:
        def prep_args(self, in_maps):
            concat_in = [
                np.concatenate([np.asarray(m[k]) for m in in_maps], axis=0)
                for k in in_names
            ]
            return [jax.device_put(a, shd) for a in concat_in]

        def exec_args(self, dev_args):
            out_arrs = sharded(*dev_args, *zeros_cat)
            jax.block_until_ready(out_arrs)
            return out_arrs

        def unpack(self, out_arrs):
            out_arrs = [np.asarray(a) for a in out_arrs]
            return [
                {
                    k: out_arrs[i].reshape(NCORES, *out_avals[i].shape)[c]
                    for i, k in enumerate(out_names)
                }
                for c in range(NCORES)
            ]

        def __call__(self, in_maps):
            return self.unpack(self.exec_args(self.prep_args(in_maps)))

    return Runner()


def _eps_host():
    """eps must bit-match what reference() computes in this environment.
    The axon container pins jax's PRNG impl to 'rbg' (JAX_PLATFORMS is
    overridden by the boot sitecustomize), so the plain in-process call
    reproduces the reference's eps exactly."""
    if "eps" not in _CACHE:
        e = jax.random.normal(jax.random.key(42), (N, LAT), jnp.float32)
        _CACHE["eps"] = np.asarray(e)
    return _CACHE["eps"]


def _prep_in_maps(x, entity_emb, time_emb, params):
    p = {k: np.asarray(v) for k, v in params.items() if not isinstance(v, dict)}
    tg = {
        name: {k: np.asarray(v) for k, v in params[name].items()}
        for name in ("tg1", "tg2", "tgd")
    }
    x = np.asarray(x, np.float32)
    ee = np.asarray(entity_emb, np.float32)
    tt = np.asarray(time_emb, np.float32)
    es = np.asarray(p["edge_score"], np.float32)

    def gates(t):
        nz = -(t["lbz"] + t["bz"] @ t["lWz"][: t["Wz"].shape[1]])
        gh = t["lbh"] + t["bh"] @ t["lWh"][: t["Wh"].shape[1]]
        return nz.astype(np.float32), gh.astype(np.float32)

    nz1, gh1 = gates(tg["tg1"])
    nz2, gh2 = gates(tg["tg2"])
    nz3, gh3 = gates(tg["tgd"])

    bia = np.zeros((P, NBIA), np.float32)

    def put(col, vec):
        v = vec.reshape(-1, P).T  # [P, chunks]
        bia[:, col : col + v.shape[1]] = v

    put(BE, p["be"])
    put(BT, p["bt"])
    put(NZ1, nz1)
    put(GH1, gh1)
    put(NZ2, nz2)
    put(GH2, gh2)
    put(NZ3, nz3)
    put(GH3, gh3)
    put(BMULV, np.concatenate([p["bmu"], p["blv"]]))
    put(BD, p["bd"])

    w1 = np.concatenate([tg["tg1"]["Wz"], tg["tg1"]["Wh"]], axis=1)
    w2 = np.concatenate([tg["tg2"]["Wz"], tg["tg2"]["Wh"]], axis=1)
    w3 = np.concatenate([tg["tgd"]["Wz"], tg["tgd"]["Wh"]], axis=1)
    wmulv = np.concatenate([p["Wmu"], p["Wlv"]], axis=1)
    wd = np.zeros((P, HID), np.float32)
    wd[:LAT] = p["Wd"]
    eps = _eps_host()

    shared = {
        "we": p["We"], "wt": p["Wt"], "w1": w1,
        "lw1z": tg["tg1"]["lWz"][:HID], "lw1h": tg["tg1"]["lWh"][:HID],
        "w2": w2, "lw2z": tg["tg2"]["lWz"][:HID], "lw2h": tg["tg2"]["lWh"][:HID],
        "wmulv": wmulv, "wd": wd, "w3": w3,
        "lw3z": tg["tgd"]["lWz"][:DIN], "lw3h": tg["tgd"]["lWh"][:DIN],
        "bia": bia,
    }
    shared = {k: np.ascontiguousarray(v, np.float32) for k, v in shared.items()}

    in_maps = []
    for c in range(NCORES):
        J = slice(S * c, S * (c + 1))
        m = dict(shared)
        m["ecol"] = np.ascontiguousarray(es[:, J])
        m["xT"] = np.ascontiguousarray(x[J].T)
        m["eeT"] = np.ascontiguousarray(ee[J].T)
        m["ttT"] = np.ascontiguousarray(tt[J].T)
        m["epsT"] = np.ascontiguousarray(eps[J].T)
        in_maps.append(m)
    return in_maps


def _get_runner():
    if "runner" not in _CACHE:
        nc = _build()
        _CACHE["runner"] = _make_runner(nc)
    return _CACHE["runner"]


def kernel(x, entity_emb, time_emb, num_nodes, params):
    in_maps = _prep_in_maps(x, entity_emb, time_emb, params)
    results = _get_runner()(in_maps)
    recon = np.concatenate([r["recon_o"] for r in results], axis=0)
    mu = np.concatenate([r["mu_o"] for r in results], axis=0)
    logvar = np.concatenate([r["lv_o"] for r in results], axis=0)
    adj = np.concatenate([r["adj_o"] for r in results], axis=1)
    return recon, mu, logvar, adj
